# revision 47
# baseline (speedup 1.0000x reference)
"""Trainium2 Bass kernel v2 for nn_DSSM_57629871178390 (dual-stream Mamba).

Sharding: d_inner=256 split 8 ways (32 ch/core). All intermediates SBUF-
resident; dt computed upfront per batch via Exp+Ln(1+x) (same act table as
the scan's Exp -> no table reloads); replication to the 128-partition scan
layout via PE select-matmuls from base-0 [32,T] sources; scan on DVE in
Tc=2048 chunks with carry chaining.

Self-contained: hardcodes all shapes; imports numpy + concourse.
"""

import sys
from dataclasses import dataclass

import numpy as np

if "/opt/trn_rl_repo" not in sys.path:
    sys.path.insert(0, "/opt/trn_rl_repo")

import ml_dtypes  # noqa: E402

import concourse.bass as bass  # noqa: E402
import concourse.bacc as bacc  # noqa: E402
import concourse.tile as tile  # noqa: E402
import concourse.mybir as mybir  # noqa: E402
from concourse import bass_utils  # noqa: E402

F32 = mybir.dt.float32
BF16 = mybir.dt.bfloat16
AF = mybir.ActivationFunctionType
OP = mybir.AluOpType
NPBF16 = ml_dtypes.bfloat16


@dataclass(frozen=True)
class Cfg:
    B: int = 2
    C: int = 128
    H: int = 64
    W: int = 64
    D: int = 256
    N: int = 16
    R: int = 8
    K: int = 3
    n_cores: int = 8
    fake_cc: bool = False
    dtx_mode: str = "dma"   # "pe": sel-matmul + 1x DVE mul from PSUM
                            # "dma": rep_sbuf DMA + 2x DVE mul
    y1_evac: str = "act"    # engine for y1 PSUM evacuation (no pool: PSUM)
    silu_lut: bool = True   # False: sigmoid+mul (CoreSim lacks Silu)
    pool_frac: int = 0      # every pool_frac-th big mul goes to Pool (0=off)
    carry_eng: str = "act"  # engine for the scan-carry column copy
    x_evac: str = "act"     # engine for x_dbl PSUM evacuation
    drps_big: bool = False  # [128,2048] dtrep PSUM + single exp per group
    dtx_eng: str = "dve"    # engine for per-chunk dtx muls
    phases: str = "fxdsl"

    @property
    def L(self):
        return self.H * self.W

    @property
    def L2(self):
        return 2 * self.L

    @property
    def Dsh(self):
        return self.D // self.n_cores

    @property
    def DG(self):
        return 128 // self.N        # 8 d per scan tile

    @property
    def G(self):
        return self.Dsh // self.DG  # 4 scan tiles per (b,k)

    @property
    def Tc(self):
        return 2048                 # scan chunk

    @property
    def TOK(self):
        return 512


CFG = Cfg()
MMF = 512


# ---------------------------------------------------------------------------
# Host-side preparation
# ---------------------------------------------------------------------------

def host_prep(cfg: Cfg, inputs: dict) -> list:
    B, C, H, W = cfg.B, cfg.C, cfg.H, cfg.W
    D, N, R, K = cfg.D, cfg.N, cfg.R, cfg.K
    Dsh, G, DG, L = cfg.Dsh, cfg.G, cfg.DG, cfg.L

    ms = np.asarray(inputs["ms"], np.float32).reshape(B, C, L)
    pan = np.asarray(inputs["pan"], np.float32).reshape(B, C, L)
    w_ms = np.asarray(inputs["in_proj_ms_w"], np.float32)
    w_pan = np.asarray(inputs["in_proj_pan_w"], np.float32)
    cw_ms = np.asarray(inputs["conv_ms_w"], np.float32)
    cb_ms = np.asarray(inputs["conv_ms_b"], np.float32)
    cw_pan = np.asarray(inputs["conv_pan_w"], np.float32)
    cb_pan = np.asarray(inputs["conv_pan_b"], np.float32)
    xpw = np.asarray(inputs["x_proj_weight"], np.float32)
    dtw = np.asarray(inputs["dt_projs_weight"], np.float32)
    dtb = np.asarray(inputs["dt_projs_bias"], np.float32)
    A_logs = np.asarray(inputs["A_logs"], np.float32)
    Ds = np.asarray(inputs["Ds"], np.float32)
    lnw_v = np.asarray(inputs["out_norm_vis_w"], np.float32)
    lnb_v = np.asarray(inputs["out_norm_vis_b"], np.float32)
    lnw_i = np.asarray(inputs["out_norm_inf_w"], np.float32)
    lnb_i = np.asarray(inputs["out_norm_inf_b"], np.float32)
    opw_ms = np.asarray(inputs["out_proj_ms_w"], np.float32)
    opw_pan = np.asarray(inputs["out_proj_pan_w"], np.float32)

    A = np.exp(A_logs).reshape(K, D, N)       # positive; sign lives in A_col
    Dsum = Ds.reshape(K, D).sum(0)

    # selg[g]: [32,128] each-rep-16 select: out p reads row 8g + p//16
    # stacked 3x so lhsT base can match rhs k-block base (0/32/64)
    selg1 = np.zeros((Dsh, G * 128), NPBF16)
    for g in range(G):
        for p in range(128):
            selg1[DG * g + p // N, g * 128 + p] = 1.0
    selg = np.concatenate([selg1, selg1, selg1], 0)

    # red8: [128, G*32]: slice g is lhsT [128,32]: col 8g + p//16
    red8 = np.zeros((128, G * Dsh), NPBF16)
    for g in range(G):
        for p in range(128):
            red8[p, g * Dsh + DG * g + p // N] = 1.0

    selst_h = np.zeros((128, 2, 4 * B), NPBF16)
    for pi in range(128):
        selst_h[pi, 0, pi // Dsh] = 1.0
        selst_h[pi, 1, 2 * B + pi // Dsh] = 1.0

    # selbc[:,0,:]: B-row replication (out p reads row p%N); [:,1,:]: C rows
    selbc_h = np.zeros((2 * N, 2 * 128), NPBF16)
    for p in range(128):
        selbc_h[p % N, p] = 1.0
        selbc_h[N + p % N, 128 + p] = 1.0

    shared = {
        "msf": ms.astype(NPBF16),
        "panf": pan.astype(NPBF16),
        "selg": selg,
        "red8": red8,
        "selst": selst_h.reshape(128, 2 * 4 * B),
        "selbc": selbc_h,
    }

    maps = []
    for c in range(cfg.n_cores):
        dsl = slice(Dsh * c, Dsh * (c + 1))
        m = dict(shared)
        # in_proj: cols 0:32 z rows, 32:64 x rows
        m["w_in_ms"] = np.ascontiguousarray(
            np.concatenate([w_ms[D + Dsh * c: D + Dsh * (c + 1)].T,
                            w_ms[dsl].T], 1)).astype(NPBF16)
        m["w_in_pan"] = np.ascontiguousarray(
            np.concatenate([w_pan[D + Dsh * c: D + Dsh * (c + 1)].T,
                            w_pan[dsl].T], 1)).astype(NPBF16)
        # conv quadrant diag: [128, 9*128]; p=(2b+s)*32+d
        cd4 = np.zeros((128, 9 * 128), NPBF16)
        cb4 = np.zeros((128, 1), np.float32)
        for q in range(4):
            s = q % 2
            cw = cw_ms if s == 0 else cw_pan
            cb = cb_ms if s == 0 else cb_pan
            for i in range(Dsh):
                p = q * Dsh + i
                cb4[p, 0] = cb[Dsh * c + i]
                for t in range(9):
                    cd4[p, t * 128 + p] = cw[Dsh * c + i, 0, t // 3, t % 3]
        m["conv_d4"] = cd4
        m["conv_b4"] = cb4
        m["xproj_T"] = np.ascontiguousarray(
            xpw[:, :, dsl].transpose(2, 0, 1).reshape(Dsh, K * (R + 2 * N))
        ).astype(NPBF16)
        # dtw block-diag [24, 96]; ndt row-blocks in scan order (k=0,2,1)
        KB = {0: 0, 2: 1, 1: 2}
        dtwp = np.zeros((K * R, K * Dsh), NPBF16)
        dtbp = np.zeros((K * Dsh, 1), np.float32)
        for k in range(K):
            dtwp[R * k:R * (k + 1), Dsh * KB[k]:Dsh * (KB[k] + 1)] = \
                dtw[k, dsl, :].T
            dtbp[Dsh * KB[k]:Dsh * (KB[k] + 1), 0] = dtb[k, dsl]
        m["dtw_pack"] = dtwp
        m["dtb_pack"] = dtbp
        acol = np.zeros((128, K * G), np.float32)
        for k in range(K):
            for g in range(G):
                for p in range(128):
                    dd, n = p // N, p % N
                    acol[p, KB[k] * G + g] = -A[k, Dsh * c + DG * g + dd, n]
        m["A_col"] = acol
        m["Dsum"] = Dsum[dsl].reshape(Dsh, 1).astype(np.float32)
        lw4 = np.zeros((128, 2), np.float32)
        for bs in range(2 * B):
            sle = bs % 2
            lw4[bs * Dsh:(bs + 1) * Dsh, 0] = (lnw_v if sle == 0 else lnw_i)[dsl]
            lw4[bs * Dsh:(bs + 1) * Dsh, 1] = (lnb_v if sle == 0 else lnb_i)[dsl]
        m["lnwb4"] = lw4
        o4 = np.zeros((128, C), NPBF16)
        for bs in range(2 * B):
            sle = bs % 2
            o4[bs * Dsh:(bs + 1) * Dsh] = (
                (opw_ms if sle == 0 else opw_pan)[:, dsl].T).astype(NPBF16)
        m["opw4"] = o4
        m["opw"] = np.ascontiguousarray(
            np.concatenate([opw_ms[:, dsl].T, opw_pan[:, dsl].T], 1)
        ).astype(NPBF16)
        maps.append(m)
    return maps


def mm(nc, out_ps, lhsT, rhs, start, stop, maxf=MMF):
    F = rhs.shape[-1]
    if F <= maxf:
        nc.tensor.matmul(out_ps, lhsT, rhs, start=start, stop=stop,
                         skip_group_check=True)
        return
    assert F % maxf == 0
    for i in range(F // maxf):
        nc.tensor.matmul(out_ps[:, i * maxf:(i + 1) * maxf], lhsT,
                         rhs[:, i * maxf:(i + 1) * maxf], start=start,
                         stop=stop, skip_group_check=True)


def rep_dram(tensor_ap, offset, row_stride, n_rows, n_rep, fstep, fcount):
    """DRAM src AP: dest p = rep*n_rows ... tiled replication
    (rep, row, f) -> [[0, n_rep], [row_stride, n_rows], [fstep, fcount]]."""
    return bass.AP(tensor=tensor_ap.tensor, offset=tensor_ap.offset + offset,
                   ap=[[0, n_rep], [row_stride, n_rows], [fstep, fcount]])


def rep_sbuf(ap2d, n_inner):
    """SBUF [P, F] -> dest [P*n_inner, F], dest p = i*n_inner+j reads src i."""
    pairs = [list(x) for x in ap2d.ap]
    assert len(pairs) == 2
    return bass.AP(tensor=ap2d.tensor, offset=ap2d.offset,
                   ap=[pairs[0], [0, n_inner], pairs[1]])


# ---------------------------------------------------------------------------
# Bass program
# ---------------------------------------------------------------------------

def build_nc(cfg: Cfg):
    B, C, H, W = cfg.B, cfg.C, cfg.H, cfg.W
    D, N, R, K = cfg.D, cfg.N, cfg.R, cfg.K
    Dsh, G, DG = cfg.Dsh, cfg.G, cfg.DG
    L, L2, Tc = cfg.L, cfg.L2, cfg.Tc
    XD = R + 2 * N                   # 40
    Hp, Wp = H + 2, W + 2
    NCH = L2 // Tc                   # 4 scan chunks
    TcL = 512                        # L-phase chunk
    nLch = L // TcL
    SC = 2 * B * L // 64
    groups = [list(range(cfg.n_cores))]
    half = 2 * B * L

    nc = bacc.Bacc("TRN2", target_bir_lowering=False, debug=False,
                   enable_asserts=False, num_devices=cfg.n_cores)

    t = {}

    def inp(name, shape, dt):
        t[name] = nc.dram_tensor(name, shape, dt, kind="ExternalInput").ap()

    inp("msf", [B, C, L], BF16)
    inp("panf", [B, C, L], BF16)
    inp("w_in_ms", [C, 2 * Dsh], BF16)
    inp("w_in_pan", [C, 2 * Dsh], BF16)
    inp("conv_d4", [128, 9 * 128], BF16)
    inp("conv_b4", [128, 1], F32)
    inp("xproj_T", [Dsh, K * XD], BF16)
    inp("dtw_pack", [K * R, K * Dsh], BF16)
    inp("dtb_pack", [K * Dsh, 1], F32)
    inp("A_col", [128, K * G], F32)
    inp("Dsum", [Dsh, 1], F32)
    inp("selg", [K * Dsh, G * 128], BF16)
    inp("red8", [128, G * Dsh], BF16)
    inp("selst", [128, 2 * 4 * B], BF16)
    inp("selbc", [2 * N, 2 * 128], BF16)
    inp("lnwb4", [128, 2], F32)
    inp("opw4", [128, C], BF16)
    inp("opw", [Dsh, 2 * C], BF16)

    out_ms = nc.dram_tensor("out_ms", [B, C, H, W], F32,
                            kind="ExternalOutput").ap()
    out_pan = nc.dram_tensor("out_pan", [B, C, H, W], F32,
                             kind="ExternalOutput").ap()

    shsp = "Shared" if cfg.n_cores > 4 else "Local"
    xdbl_part = nc.dram_tensor("xdbl_part", [B, K * XD, L2], BF16,
                               kind="Internal").ap()
    xdbl_full = nc.dram_tensor("xdbl_full", [B, K * XD, L2], BF16,
                               kind="Internal", addr_space=shsp).ap()
    brc_dram = nc.dram_tensor("brc_dram", [2 * N, B, L2], BF16,
                              kind="Internal").ap()
    stats_part = nc.dram_tensor("stats_part", [2, 2 * B, L], F32,
                                kind="Internal").ap()
    stats_full = nc.dram_tensor("stats_full", [2, 2 * B, L], F32,
                                kind="Internal", addr_space=shsp).ap()
    ab_dram = nc.dram_tensor("ab_dram", [2, 2 * B, L], BF16,
                             kind="Internal").ap()

    def sb(name, shape, dt):
        return nc.alloc_sbuf_tensor(name, shape, dt).ap()

    w_in_ms = sb("w_in_ms_s", [C, 2 * Dsh], BF16)
    w_in_pan = sb("w_in_pan_s", [C, 2 * Dsh], BF16)
    conv_d4 = sb("conv_d4_s", [128, 9, 128], BF16)
    conv_b4 = sb("conv_b4_s", [128, 1], F32)
    xproj_T = sb("xproj_T_s", [Dsh, K, XD], BF16)
    dtw_pack = sb("dtw_pack_s", [K * R, K * Dsh], BF16)
    dtb_pack = sb("dtb_pack_s", [K * Dsh, 1], F32)
    A_col = sb("A_col_s", [128, K * G], F32)
    Dsum_s = sb("Dsum_s", [Dsh, 1], F32)
    selg_s = sb("selg_s", [K * Dsh, G, 128], BF16)
    red8_s = sb("red8_s", [128, G, Dsh], BF16)
    selst = sb("selst_s", [128, 2, 4 * B], BF16)
    selbc = sb("selbc_s", [2 * N, 2, 128], BF16)
    lnwb4 = sb("lnwb4_s", [128, 2], F32)
    opw4 = sb("opw4_s", [128, C], BF16)
    opw_s = sb("opw_s", [Dsh, 2, C], BF16)

    zs4 = sb("zs4_s", [128, L], BF16)
    carry = sb("carry_s", [128, K * G], F32)
    ysv = sb("ysv_s", [Dsh, L2], BF16)
    y1v = sb("y1v_s", [Dsh, L2], BF16)

    def allreduce(in_ap, out_ap):
        if cfg.fake_cc:
            nc.sync.dma_start(out=out_ap, in_=in_ap)
        else:
            nc.gpsimd.collective_compute(
                "AllReduce", OP.add, replica_groups=groups,
                ins=[in_ap.opt()], outs=[out_ap.opt()])

    with tile.TileContext(nc) as tc:
        from contextlib import ExitStack
        # ---- weight loads ----
        for dst, srcw in [
            (w_in_ms, t["w_in_ms"]), (w_in_pan, t["w_in_pan"]),
            (conv_d4, t["conv_d4"].rearrange("p (t q) -> p t q", q=128)),
            (conv_b4, t["conv_b4"]),
            (xproj_T, t["xproj_T"].rearrange("p (k x) -> p k x", x=XD)),
            (dtw_pack, t["dtw_pack"]), (dtb_pack, t["dtb_pack"]),
            (A_col, t["A_col"]), (Dsum_s, t["Dsum"]),
            (selg_s, t["selg"].rearrange("p (g q) -> p g q", q=128)),
            (red8_s, t["red8"].rearrange("p (j q) -> p j q", q=Dsh)),
            (selst, t["selst"].rearrange("p (x m) -> p x m", m=4 * B)),
            (selbc, t["selbc"].rearrange("p (x q) -> p x q", q=128)),
            (lnwb4, t["lnwb4"]), (opw4, t["opw4"]),
            (opw_s, t["opw"].rearrange("p (s c) -> p s c", c=C)),
        ]:
            nc.sync.dma_start(out=dst, in_=srcw)

        ydp_ctx = ExitStack()
        ydp_pool = ydp_ctx.enter_context(tc.tile_pool(name="ydpp", bufs=1))
        ydp = ydp_pool.tile([128, L], BF16, tag="ydp")
        big_ctx = ExitStack()
        big = big_ctx.enter_context(tc.tile_pool(name="big", bufs=1))
        inter = big.tile([Dsh, B, L2], BF16, tag="inter")
        ndt = big.tile([K * Dsh, B, L2], BF16, tag="ndt")

        # ================= Phase F: in_proj + conv + silu =================
        if "f" in cfg.phases:
          with tc.tile_pool(name="f_ps", bufs=1, space="PSUM") as f_ps, \
               tc.tile_pool(name="f_cv", bufs=2, space="PSUM") as f_cv, \
               tc.tile_pool(name="f_src", bufs=2) as f_src, \
               tc.tile_pool(name="f_mt", bufs=1) as f_mt, \
               tc.tile_pool(name="f_pad", bufs=1) as f_pad:
            xpad4 = f_pad.tile([128, Hp, Wp], BF16, tag="xpad4")
            nc.vector.memset(xpad4, 0.0)
            TOK = cfg.TOK
            mts = []
            for q in range(4):
                b, s = q // 2, q % 2
                srcT = t["msf"] if s == 0 else t["panf"]
                mt = f_mt.tile([C, L], BF16, tag=f"msrc{q}")
                nc.sync.dma_start(out=mt, in_=srcT[b])
                mts.append(mt)
            rpc = TOK // W
            for j in range(L // TOK):
                js = slice(j * TOK, (j + 1) * TOK)
                # quadrant pairs packed into [64,TOK] PSUM tiles (PE out
                # base must be 0/32/64): one silu / copy per pair
                for h in range(2):
                    ps_z = f_ps.tile([2 * Dsh, TOK], F32, tag=f"psz{h}")
                    ps_x = f_ps.tile([2 * Dsh, TOK], F32, tag=f"psx{h}")
                    for qq in range(2):
                        q = 2 * h + qq
                        w_in = w_in_ms if q % 2 == 0 else w_in_pan
                        qs = slice(qq * Dsh, (qq + 1) * Dsh)
                        mm(nc, ps_z[qs, :], w_in[:, 0:Dsh],
                           mts[q][:, js], start=True, stop=True)
                        mm(nc, ps_x[qs, :], w_in[:, Dsh:2 * Dsh],
                           mts[q][:, js], start=True, stop=True)
                    zh = slice(2 * h * Dsh, (2 * h + 2) * Dsh)
                    if h == 0:
                        nc.scalar.activation(out=zs4[zh, js], in_=ps_z,
                                             func=AF.Silu)
                    else:
                        zstg = f_src.tile([2 * Dsh, TOK], BF16, tag="zstg")
                        nc.scalar.activation(out=zstg, in_=ps_z,
                                             func=AF.Silu)
                        nc.vector.tensor_copy(out=zs4[zh, js], in_=zstg)
                    nc.vector.tensor_copy(
                        out=xpad4[zh, 1 + j * rpc:1 + (j + 1) * rpc,
                                  1:1 + W],
                        in_=ps_x.rearrange("p (r w) -> p r w", w=W))
            # conv: all 4 quadrants at once, 9 taps, 8 row-chunks
            CRW = 8
            for j in range(H // CRW):
                cps = f_cv.tile([128, CRW * W], F32, tag="cps")
                for tap in range(9):
                    ky, kx = tap // 3, tap % 3
                    rhs = xpad4[:, ky + j * CRW: ky + (j + 1) * CRW,
                                kx:kx + W]
                    nc.tensor.matmul(cps, conv_d4[:, tap, :], rhs,
                                     start=(tap == 0), stop=(tap == 8),
                                     skip_group_check=True)
                # silu(+bias) full width, then DVE strided copies to inter
                stg4 = f_src.tile([128, CRW * W], BF16, tag="stg4")
                if cfg.silu_lut:
                    nc.scalar.activation(out=stg4, in_=cps, func=AF.Silu,
                                         bias=conv_b4)
                else:
                    sgc = f_src.tile([128, CRW * W], BF16, tag="sgc")
                    nc.scalar.activation(out=sgc, in_=cps, func=AF.Sigmoid,
                                         bias=conv_b4)
                    nc.vector.scalar_tensor_tensor(
                        out=stg4, in0=cps, scalar=conv_b4, in1=sgc,
                        op0=OP.add, op1=OP.mult)
                for q in range(4):
                    b, s = q // 2, q % 2
                    base = inter[:, b, :]
                    ppair = list(base.ap[0])
                    dst = bass.AP(
                        tensor=base.tensor,
                        offset=base.offset + 2 * (j * CRW * W) + s,
                        ap=[ppair, [2, CRW * W]])
                    nc.vector.tensor_copy(
                        out=dst, in_=stg4[q * Dsh:(q + 1) * Dsh, :])
        # ================= Phase X: x_dbl partial + AllReduce ============
        if "x" in cfg.phases:
            with tc.tile_pool(name="x_ps", bufs=2, space="PSUM") as x_ps, \
                 tc.tile_pool(name="x_st", bufs=3) as x_st:
                for b in range(B):
                    for k in range(K):
                        for jj in range(L2 // Tc):
                            ps = x_ps.tile([XD, Tc], F32, tag="xps")
                            mm(nc, ps, xproj_T[:, k, :],
                               inter[:, b, jj * Tc:(jj + 1) * Tc],
                               start=True, stop=True)
                            st = x_st.tile([XD, Tc], BF16, tag="xst")
                            if cfg.x_evac == "pool":
                                nc.gpsimd.tensor_copy(out=st, in_=ps)
                            elif cfg.x_evac == "dve":
                                nc.vector.tensor_copy(out=st, in_=ps)
                            elif cfg.x_evac == "mix":
                                if (k + jj) % 2 == 0:
                                    nc.scalar.copy(out=st, in_=ps)
                                else:
                                    nc.vector.tensor_copy(out=st, in_=ps)
                            else:
                                nc.scalar.copy(out=st, in_=ps)
                            nc.sync.dma_start(
                                out=xdbl_part[b, k * XD:(k + 1) * XD,
                                              jj * Tc:(jj + 1) * Tc],
                                in_=st)
                    # per-b collective: b=0's AllReduce overlaps b=1's matmuls
                    allreduce(xdbl_part[b], xdbl_full[b])

        # ================= Phase D: dt / dtx / reversals per b ===========
        if "d" in cfg.phases:
          with tc.tile_pool(name="d_ps", bufs=2, space="PSUM") as d_ps, \
               tc.tile_pool(name="d_xdt", bufs=1) as d_xdt, \
               tc.tile_pool(name="d_e", bufs=2) as d_e, \
               tc.tile_pool(name="d_bc", bufs=2) as d_bc:
            for b in range(B):
                xdt = d_xdt.tile([K * R, L2], BF16, tag="xdt")
                for k in range(K):
                    nc.sync.dma_start(
                        out=xdt[k * R:(k + 1) * R, :],
                        in_=xdbl_full[b, k * XD:k * XD + R, :])
                for jj in range(L2 // Tc):
                    ps = d_ps.tile([K * Dsh, Tc], F32, tag="dps")
                    mm(nc, ps, dtw_pack, xdt[:, jj * Tc:(jj + 1) * Tc],
                       start=True, stop=True)
                    e_t = d_e.tile([K * Dsh, Tc], F32, tag="et")
                    nc.scalar.activation(out=e_t, in_=ps, func=AF.Exp,
                                         bias=dtb_pack)
                    nc.scalar.activation(
                        out=ndt[:, b, jj * Tc:(jj + 1) * Tc], in_=e_t,
                        func=AF.Ln, bias=1.0)
                # reversed B/C rows for k=1 -> brc_dram (chunked)
                for jj in range(L2 // Tc):
                    bc1 = d_bc.tile([2 * N, Tc], BF16, tag="bc1")
                    nc.sync.dma_start(
                        out=bc1,
                        in_=xdbl_full[b, XD + R:XD + R + 2 * N,
                                      L2 - (jj + 1) * Tc:L2 - jj * Tc])
                    bcR = d_bc.tile([2 * N, Tc], BF16, tag="bcR")
                    nc.vector.tensor_copy(out=bcR, in_=bc1[:, ::-1])
                    nc.sync.dma_start(
                        out=brc_dram[:, b, jj * Tc:(jj + 1) * Tc], in_=bcR)

        # ================= Phase S: selective scan ========================
        if "s" in cfg.phases:
          with tc.tile_pool(name="s_y", bufs=1, space="PSUM") as s_y, \
               tc.tile_pool(name="s_dr", bufs=2, space="PSUM") as s_dr, \
               tc.tile_pool(name="s_bc", bufs=2, space="PSUM") as s_bc, \
               tc.tile_pool(name="s_a", bufs=2) as s_a, \
               tc.tile_pool(name="s_b", bufs=2) as s_b, \
               tc.tile_pool(name="s_h", bufs=2) as s_h, \
               tc.tile_pool(name="s_hc", bufs=2) as s_hc, \
               tc.tile_pool(name="s_rep", bufs=2) as s_rep, \
               tc.tile_pool(name="s_xr", bufs=4) as s_xr, \
               tc.tile_pool(name="s_bcin", bufs=2) as s_bcin, \
               tc.tile_pool(name="s_io", bufs=2) as s_io, \
               tc.tile_pool(name="s_ior", bufs=1) as s_ior:
            mulctr = [0]

            def big_mul(out, in0, in1):
                mulctr[0] += 1
                if cfg.pool_frac and mulctr[0] % cfg.pool_frac == 0:
                    nc.gpsimd.tensor_tensor(out=out, in0=in0, in1=in1,
                                            op=OP.mult)
                else:
                    nc.vector.tensor_tensor(out=out, in0=in0, in1=in1,
                                            op=OP.mult)

            for b in range(B):
                for ch in range(NCH):
                    cs = slice(ch * Tc, (ch + 1) * Tc)
                    y_ps = s_y.tile([Dsh, Tc], F32, tag="yps")
                    for k in (0, 2, 1):
                        if k == 1:
                            # y02 complete: evac y0+y2+D*x to SBUF staging
                            nc.vector.scalar_tensor_tensor(
                                out=ysv[:, cs], in0=inter[:, b, cs],
                                scalar=Dsum_s[:, 0:1], in1=y_ps,
                                op0=OP.mult, op1=OP.add)
                            y_ps = s_y.tile([Dsh, Tc], F32, tag="yps")
                        # B/C rows: one linear [2N,Tc] load, then PE
                        # select-matmul replication (cheap vs rep-DMA)
                        bct = s_bcin.tile([2 * N, Tc], BF16, tag="bct")
                        if k == 1:
                            nc.gpsimd.dma_start(
                                out=bct, in_=brc_dram[:, b, cs])
                        else:
                            nc.gpsimd.dma_start(
                                out=bct,
                                in_=xdbl_full[b, k * XD + R:
                                              k * XD + R + 2 * N, cs])
                        blk = {0: 0, 2: 1, 1: 2}[k]
                        kb = 0 if k == 1 else blk * Dsh
                        if k == 1:
                            mcs = slice(L2 - (ch + 1) * Tc, L2 - ch * Tc)
                            src_dt = s_ior.tile([Dsh, Tc], BF16, tag="ndtRc")
                            nc.vector.tensor_copy(
                                out=src_dt,
                                in_=ndt[2 * Dsh:3 * Dsh, b, mcs][:, ::-1])
                            src_dtx = s_io.tile([Dsh, Tc], BF16, tag="dtxc")
                            nc.vector.tensor_tensor(
                                out=src_dtx, in0=src_dt,
                                in1=inter[:, b, mcs][:, ::-1], op=OP.mult)
                        else:
                            src_dt = ndt[blk * Dsh:(blk + 1) * Dsh, b, cs]
                            dt0 = src_dt
                            if blk != 0:
                                dt0 = s_ior.tile([Dsh, Tc], BF16, tag="dt0")
                                nc.vector.tensor_copy(out=dt0, in_=src_dt)
                            src_dtx = s_io.tile([Dsh, Tc], BF16, tag="dtxc")
                            nc.vector.tensor_tensor(
                                out=src_dtx, in0=dt0,
                                in1=inter[:, b, cs], op=OP.mult)
                        # issue all dtx replication DMAs up front so the
                        # g-loop never waits on them
                        dtxreps = []
                        for g in range(G):
                            dtxrep = s_xr.tile([128, Tc], BF16,
                                               tag="dtxrep")
                            nc.sync.dma_start(
                                out=dtxrep,
                                in_=rep_sbuf(src_dtx[g * DG:(g + 1) * DG],
                                             N))
                            dtxreps.append(dtxrep)
                        B_rep = s_rep.tile([128, Tc], BF16, tag="brep")
                        C_rep = s_rep.tile([128, Tc], BF16, tag="crep")
                        for sub in range(Tc // MMF):
                            ss = slice(sub * MMF, (sub + 1) * MMF)
                            bps = s_bc.tile([128, MMF], F32, tag="bcps")
                            nc.tensor.matmul(
                                bps, selbc[:, 0, :], bct[:, ss],
                                start=True, stop=True,
                                skip_group_check=True)
                            nc.scalar.copy(out=B_rep[:, ss], in_=bps)
                            cps2 = s_bc.tile([128, MMF], F32, tag="bcps")
                            nc.tensor.matmul(
                                cps2, selbc[:, 1, :], bct[:, ss],
                                start=True, stop=True,
                                skip_group_check=True)
                            nc.scalar.copy(out=C_rep[:, ss], in_=cps2)
                        for g in range(G):
                            ci = blk * G + g
                            # a = exp(A * dt_rep); b = dtx_rep * B_rep
                            a_t = s_a.tile([128, Tc], BF16, tag="a")
                            b_t = s_b.tile([128, Tc], BF16, tag="b")
                            dtxrep = dtxreps[g]
                            if cfg.drps_big:
                                drps = s_dr.tile([128, Tc], F32, tag="drps")
                                mm(nc, drps, selg_s[kb:kb + Dsh, g, :],
                                   src_dt, start=True, stop=True)
                                nc.scalar.activation(
                                    out=a_t, in_=drps, func=AF.Exp,
                                    scale=A_col[:, ci:ci + 1])
                            else:
                                for sub in range(Tc // MMF):
                                    ss = slice(sub * MMF, (sub + 1) * MMF)
                                    drps = s_dr.tile([128, MMF], F32,
                                                     tag="drps")
                                    nc.tensor.matmul(
                                        drps, selg_s[kb:kb + Dsh, g, :],
                                        src_dt[:, ss],
                                        start=True, stop=True,
                                        skip_group_check=True)
                                    nc.scalar.activation(
                                        out=a_t[:, ss], in_=drps,
                                        func=AF.Exp,
                                        scale=A_col[:, ci:ci + 1])
                            big_mul(b_t, dtxrep, B_rep)
                            # scan
                            h_t = s_h.tile([128, Tc], BF16, tag="h")
                            init = 0.0 if ch == 0 else carry[:, ci:ci + 1]
                            nc.vector.tensor_tensor_scan(
                                h_t, a_t, b_t, init, OP.mult, OP.add)
                            if ch < NCH - 1:
                                if cfg.carry_eng == "pool":
                                    nc.gpsimd.tensor_copy(
                                        out=carry[:, ci:ci + 1],
                                        in_=h_t[:, Tc - 1:Tc])
                                else:
                                    nc.scalar.copy(
                                        out=carry[:, ci:ci + 1],
                                        in_=h_t[:, Tc - 1:Tc])
                            # hc = h * C_rep ; reduce over n into y_ps
                            hc = s_hc.tile([128, Tc], BF16, tag="hc")
                            big_mul(hc, h_t, C_rep)
                            mm(nc, y_ps, red8_s[:, g, :], hc,
                               start=(g == 0 and k in (0, 1)),
                               stop=(g == G - 1 and k in (2, 1)))
                    # y1 evac (base 0, no partition shift)
                    if cfg.y1_evac == "pool":
                        nc.gpsimd.tensor_copy(out=y1v[:, cs], in_=y_ps)
                    elif cfg.y1_evac == "act":
                        nc.scalar.copy(out=y1v[:, cs], in_=y_ps)
                    else:
                        nc.vector.tensor_copy(out=y1v[:, cs], in_=y_ps)
                # merge y02 with flipped y1 into ydp (inputs both base-0)
                for jj in range(NCH):
                    cs2 = slice(jj * Tc, (jj + 1) * Tc)
                    y1t = y1v[:, L2 - (jj + 1) * Tc:L2 - jj * Tc]
                    tok = slice(jj * Tc // 2, (jj + 1) * Tc // 2)
                    for par in range(2):
                        qs = (2 * b + par) * Dsh
                        nc.vector.tensor_tensor(
                            out=ydp[qs:qs + Dsh, tok],
                            in0=ysv[:, cs2][:, par::2],
                            in1=y1t[:, ::-1][:, par::2], op=OP.add)

        big_ctx.close()

        # ================= Phase L: LN + gate + out_proj ==================
        if "l" in cfg.phases:
          with tc.tile_pool(name="l_ps", bufs=2, space="PSUM") as l_ps, \
               tc.tile_pool(name="l_one", bufs=1) as l_one, \
               tc.tile_pool(name="l_sq", bufs=3) as l_sq, \
               tc.tile_pool(name="l_z", bufs=2 * 8) as l_z, \
               tc.tile_pool(name="l_stg", bufs=4) as l_stg:
            zwps, bzps = [], []
            for j in range(nLch):
                js = slice(j * TcL, (j + 1) * TcL)
                sqp = l_sq.tile([128, TcL], BF16, tag="sqp")
                nc.vector.tensor_tensor(out=sqp, in0=ydp[:, js],
                                        in1=ydp[:, js], op=OP.mult)
                sp = l_ps.tile([4 * B, TcL], F32, tag="sps")
                mm(nc, sp, selst[:, 0, :], ydp[:, js], start=True, stop=False)
                mm(nc, sp, selst[:, 1, :], sqp, start=False, stop=True)
                stg = l_stg.tile([4 * B, TcL], F32, tag="sstg2")
                nc.scalar.copy(out=stg, in_=sp)
                nc.sync.dma_start(
                    out=stats_part.rearrange("a x l -> (a x) l")[:, js],
                    in_=stg)
                # z-gate pre-products: independent of stats, fill DVE
                # while the stats AllReduce runs
                zwp = l_z.tile([128, TcL], BF16, tag="zwp")
                bzp = l_z.tile([128, TcL], BF16, tag="bzp")
                nc.vector.tensor_scalar_mul(zwp, zs4[:, js], lnwb4[:, 0:1])
                nc.vector.tensor_scalar_mul(bzp, zs4[:, js], lnwb4[:, 1:2])
                zwps.append(zwp)
                bzps.append(bzp)
            allreduce(stats_part, stats_full)
            s1f = l_one.tile([64, SC], F32, tag="s1f")
            s2f = l_one.tile([64, SC], F32, tag="s2f")
            flat = stats_full.rearrange("a x l -> (a x l)")
            nc.sync.dma_start(
                out=s1f, in_=flat[0:half].rearrange("(p c) -> p c", p=64))
            nc.sync.dma_start(
                out=s2f, in_=flat[half:2 * half].rearrange(
                    "(p c) -> p c", p=64))
            mu_t = l_one.tile([64, SC], F32, tag="mu_t")
            var_t = l_one.tile([64, SC], F32, tag="var_t")
            musq = l_one.tile([64, SC], F32, tag="musq")
            eps_t = l_one.tile([64, 1], F32, tag="eps_t")
            nc.vector.memset(eps_t, 1e-5)
            nc.vector.tensor_scalar_mul(mu_t, s1f, 1.0 / D)
            nc.vector.tensor_scalar_mul(var_t, s2f, 1.0 / D)
            nc.vector.tensor_mul(musq, mu_t, mu_t)
            nc.vector.tensor_sub(var_t, var_t, musq)
            nc.scalar.activation(out=var_t, in_=var_t, func=AF.Sqrt,
                                 bias=eps_t)
            nc.vector.reciprocal(out=s1f, in_=var_t)
            nc.vector.tensor_mul(s2f, mu_t, s1f)
            nc.vector.tensor_scalar_mul(s2f, s2f, -1.0)
            s1h = l_one.tile([64, SC], BF16, tag="s1h")
            s2h = l_one.tile([64, SC], BF16, tag="s2h")
            nc.vector.tensor_copy(out=s1h, in_=s1f)
            nc.vector.tensor_copy(out=s2h, in_=s2f)
            nc.sync.dma_start(
                out=ab_dram.rearrange("a x l -> (a x l)")[0:half].rearrange(
                    "(p c) -> p c", p=64), in_=s1h)
            nc.sync.dma_start(
                out=ab_dram.rearrange("a x l -> (a x l)")[
                    half:2 * half].rearrange("(p c) -> p c", p=64), in_=s2h)

            with tc.tile_pool(name="l_rep", bufs=3) as l_rep, \
                 tc.tile_pool(name="l_t", bufs=3) as l_t, \
                 tc.tile_pool(name="o_st", bufs=2) as o_st, \
                 tc.tile_pool(name="o_ps", bufs=2, space="PSUM") as o_ps:
                for j in range(nLch):
                    js = slice(j * TcL, (j + 1) * TcL)
                    zwp, bzp = zwps[j], bzps[j]
                    arp = l_rep.tile([128, TcL], BF16, tag="arp")
                    brp = l_rep.tile([128, TcL], BF16, tag="brp")
                    nc.sync.dma_start(
                        out=arp,
                        in_=bass.AP(tensor=ab_dram.tensor,
                                    offset=ab_dram.offset + j * TcL,
                                    ap=[[L, 2 * B], [0, Dsh], [1, TcL]]))
                    nc.scalar.dma_start(
                        out=brp,
                        in_=bass.AP(tensor=ab_dram.tensor,
                                    offset=ab_dram.offset + half + j * TcL,
                                    ap=[[L, 2 * B], [0, Dsh], [1, TcL]]))
                    t1 = l_t.tile([128, TcL], BF16, tag="t1")
                    nc.vector.tensor_mul(t1, ydp[:, js], arp)
                    t2 = l_t.tile([128, TcL], BF16, tag="t2")
                    nc.vector.tensor_add(t2, t1, brp)
                    t3 = l_t.tile([128, TcL], BF16, tag="t3")
                    nc.vector.tensor_mul(t3, t2, zwp)
                    fgp = l_t.tile([128, TcL], BF16, tag="fgp")
                    nc.vector.tensor_add(fgp, t3, bzp)
                    for bs in range(2 * B):
                        b, s = bs // 2, bs % 2
                        qs = slice(bs * Dsh, (bs + 1) * Dsh)
                        if bs < 3:
                            lhs = opw4[qs, :]
                            rhs = fgp[qs, :]
                        else:
                            stq = o_st.tile([Dsh, TcL], BF16, tag="stq")
                            nc.vector.tensor_copy(out=stq, in_=fgp[qs, :])
                            lhs = opw_s[:, s, :]
                            rhs = stq
                        ops = o_ps.tile([C, TcL], F32, tag="ops")
                        mm(nc, ops, lhs, rhs, start=True, stop=True)
                        ost = o_st.tile([C, TcL], BF16, tag="ost")
                        nc.scalar.copy(out=ost, in_=ops)
                        # partial output (summed across cores on the host)
                        dst = out_ms if s == 0 else out_pan
                        nc.gpsimd.dma_start(
                            out=dst[b].rearrange("c h w -> c (h w)")[:, js],
                            in_=ost)
        ydp_ctx.close()

    nc.compile()
    return nc


# ---------------------------------------------------------------------------

_CACHE = {}


def _get_nc(cfg: Cfg):
    if cfg not in _CACHE:
        _CACHE[cfg] = build_nc(cfg)
    return _CACHE[cfg]


def kernel(**inputs):
    cfg = CFG
    nc = _get_nc(cfg)
    in_maps = host_prep(cfg, inputs)
    res = bass_utils.run_bass_kernel_spmd(
        nc, in_maps, core_ids=list(range(cfg.n_cores)))
    return assemble_outputs(cfg, res.results)


def assemble_outputs(cfg, results):
    # each core emits a partial (its d_inner shard's contribution); sum.
    out_ms = np.zeros((cfg.B, cfg.C, cfg.H, cfg.W), np.float32)
    out_pan = np.zeros_like(out_ms)
    for r in range(cfg.n_cores):
        out_ms += np.asarray(results[r]["out_ms"], np.float32)
        out_pan += np.asarray(results[r]["out_pan"], np.float32)
    return (out_ms, out_pan)



# revision 56
# speedup vs baseline: 1.1236x; 1.1236x over previous
"""Trainium2 Bass kernel v2 for nn_DSSM_57629871178390 (dual-stream Mamba).

Sharding: d_inner=256 split 8 ways (32 ch/core). All intermediates SBUF-
resident; dt computed upfront per batch via Exp+Ln(1+x) (same act table as
the scan's Exp -> no table reloads); replication to the 128-partition scan
layout via PE select-matmuls from base-0 [32,T] sources; scan on DVE in
Tc=2048 chunks with carry chaining.

Self-contained: hardcodes all shapes; imports numpy + concourse.
"""

import sys
from dataclasses import dataclass

import numpy as np

if "/opt/trn_rl_repo" not in sys.path:
    sys.path.insert(0, "/opt/trn_rl_repo")

import ml_dtypes  # noqa: E402

import concourse.bass as bass  # noqa: E402
import concourse.bacc as bacc  # noqa: E402
import concourse.tile as tile  # noqa: E402
import concourse.mybir as mybir  # noqa: E402
from concourse import bass_utils  # noqa: E402

F32 = mybir.dt.float32
BF16 = mybir.dt.bfloat16
AF = mybir.ActivationFunctionType
OP = mybir.AluOpType
NPBF16 = ml_dtypes.bfloat16


@dataclass(frozen=True)
class Cfg:
    B: int = 2
    C: int = 128
    H: int = 64
    W: int = 64
    D: int = 256
    N: int = 16
    R: int = 8
    K: int = 3
    n_cores: int = 8
    fake_cc: bool = False
    dtx_mode: str = "dma"   # "pe": sel-matmul + 1x DVE mul from PSUM
                            # "dma": rep_sbuf DMA + 2x DVE mul
    y1_evac: str = "act"    # engine for y1 PSUM evacuation (no pool: PSUM)
    silu_lut: bool = True   # False: sigmoid+mul (CoreSim lacks Silu)
    pool_frac: int = 0      # every pool_frac-th big mul goes to Pool (0=off)
    carry_eng: str = "act"  # engine for the scan-carry column copy
    x_evac: str = "act"     # engine for x_dbl PSUM evacuation
    drps_big: bool = False  # [128,2048] dtrep PSUM + single exp per group
    dtx_eng: str = "dve"    # engine for per-chunk dtx muls
    phases: str = "fxdsl"

    @property
    def L(self):
        return self.H * self.W

    @property
    def L2(self):
        return 2 * self.L

    @property
    def Dsh(self):
        return self.D // self.n_cores

    @property
    def DG(self):
        return 128 // self.N        # 8 d per scan tile

    @property
    def G(self):
        return self.Dsh // self.DG  # 4 scan tiles per (b,k)

    @property
    def Tc(self):
        return 2048                 # scan chunk

    @property
    def TOK(self):
        return 512


CFG = Cfg()
MMF = 512


# ---------------------------------------------------------------------------
# Host-side preparation
# ---------------------------------------------------------------------------

def host_prep(cfg: Cfg, inputs: dict) -> list:
    B, C, H, W = cfg.B, cfg.C, cfg.H, cfg.W
    D, N, R, K = cfg.D, cfg.N, cfg.R, cfg.K
    Dsh, G, DG, L = cfg.Dsh, cfg.G, cfg.DG, cfg.L

    ms = np.asarray(inputs["ms"], np.float32).reshape(B, C, L)
    pan = np.asarray(inputs["pan"], np.float32).reshape(B, C, L)
    w_ms = np.asarray(inputs["in_proj_ms_w"], np.float32)
    w_pan = np.asarray(inputs["in_proj_pan_w"], np.float32)
    cw_ms = np.asarray(inputs["conv_ms_w"], np.float32)
    cb_ms = np.asarray(inputs["conv_ms_b"], np.float32)
    cw_pan = np.asarray(inputs["conv_pan_w"], np.float32)
    cb_pan = np.asarray(inputs["conv_pan_b"], np.float32)
    xpw = np.asarray(inputs["x_proj_weight"], np.float32)
    dtw = np.asarray(inputs["dt_projs_weight"], np.float32)
    dtb = np.asarray(inputs["dt_projs_bias"], np.float32)
    A_logs = np.asarray(inputs["A_logs"], np.float32)
    Ds = np.asarray(inputs["Ds"], np.float32)
    lnw_v = np.asarray(inputs["out_norm_vis_w"], np.float32)
    lnb_v = np.asarray(inputs["out_norm_vis_b"], np.float32)
    lnw_i = np.asarray(inputs["out_norm_inf_w"], np.float32)
    lnb_i = np.asarray(inputs["out_norm_inf_b"], np.float32)
    opw_ms = np.asarray(inputs["out_proj_ms_w"], np.float32)
    opw_pan = np.asarray(inputs["out_proj_pan_w"], np.float32)

    A = np.exp(A_logs).reshape(K, D, N)       # positive; sign lives in A_col
    Dsum = Ds.reshape(K, D).sum(0)

    # selg[g]: [32,128] each-rep-16 select: out p reads row 8g + p//16
    # stacked 3x so lhsT base can match rhs k-block base (0/32/64)
    selg1 = np.zeros((Dsh, G * 128), NPBF16)
    for g in range(G):
        for p in range(128):
            selg1[DG * g + p // N, g * 128 + p] = 1.0
    selg = np.concatenate([selg1, selg1, selg1], 0)

    # red8: [128, G*32]: slice g is lhsT [128,32]: col 8g + p//16
    red8 = np.zeros((128, G * Dsh), NPBF16)
    for g in range(G):
        for p in range(128):
            red8[p, g * Dsh + DG * g + p // N] = 1.0

    selst_h = np.zeros((128, 2, 4 * B), NPBF16)
    for pi in range(128):
        selst_h[pi, 0, pi // Dsh] = 1.0
        selst_h[pi, 1, 2 * B + pi // Dsh] = 1.0

    # selbc[:,0,:]: B-row replication (out p reads row p%N); [:,1,:]: C rows
    selbc_h = np.zeros((2 * N, 2 * 128), NPBF16)
    for p in range(128):
        selbc_h[p % N, p] = 1.0
        selbc_h[N + p % N, 128 + p] = 1.0

    shared = {
        "msf": ms.astype(NPBF16),
        "panf": pan.astype(NPBF16),
        "selg": selg,
        "red8": red8,
        "selst": selst_h.reshape(128, 2 * 4 * B),
        "selbc": selbc_h,
    }

    maps = []
    for c in range(cfg.n_cores):
        dsl = slice(Dsh * c, Dsh * (c + 1))
        m = dict(shared)
        # in_proj: cols 0:32 z rows, 32:64 x rows
        m["w_in_ms"] = np.ascontiguousarray(
            np.concatenate([w_ms[D + Dsh * c: D + Dsh * (c + 1)].T,
                            w_ms[dsl].T], 1)).astype(NPBF16)
        m["w_in_pan"] = np.ascontiguousarray(
            np.concatenate([w_pan[D + Dsh * c: D + Dsh * (c + 1)].T,
                            w_pan[dsl].T], 1)).astype(NPBF16)
        # conv quadrant diag: [128, 9*128]; p=(2b+s)*32+d
        cd4 = np.zeros((128, 9 * 128), NPBF16)
        cb4 = np.zeros((128, 1), np.float32)
        for q in range(4):
            s = q % 2
            cw = cw_ms if s == 0 else cw_pan
            cb = cb_ms if s == 0 else cb_pan
            for i in range(Dsh):
                p = q * Dsh + i
                cb4[p, 0] = cb[Dsh * c + i]
                for t in range(9):
                    cd4[p, t * 128 + p] = cw[Dsh * c + i, 0, t // 3, t % 3]
        m["conv_d4"] = cd4
        m["conv_b4"] = cb4
        m["xproj_T"] = np.ascontiguousarray(
            xpw[:, :, dsl].transpose(2, 0, 1).reshape(Dsh, K * (R + 2 * N))
        ).astype(NPBF16)
        # dtw block-diag [24, 96]; ndt row-blocks in scan order (k=0,2,1)
        KB = {0: 0, 2: 1, 1: 2}
        dtwp = np.zeros((K * R, K * Dsh), NPBF16)
        dtbp = np.zeros((K * Dsh, 1), np.float32)
        for k in range(K):
            dtwp[R * k:R * (k + 1), Dsh * KB[k]:Dsh * (KB[k] + 1)] = \
                dtw[k, dsl, :].T
            dtbp[Dsh * KB[k]:Dsh * (KB[k] + 1), 0] = dtb[k, dsl]
        m["dtw_pack"] = dtwp
        m["dtb_pack"] = dtbp
        acol = np.zeros((128, K * G), np.float32)
        for k in range(K):
            for g in range(G):
                for p in range(128):
                    dd, n = p // N, p % N
                    acol[p, KB[k] * G + g] = -A[k, Dsh * c + DG * g + dd, n]
        m["A_col"] = acol
        m["Dsum"] = Dsum[dsl].reshape(Dsh, 1).astype(np.float32)
        lw4 = np.zeros((128, 2), np.float32)
        for bs in range(2 * B):
            sle = bs % 2
            lw4[bs * Dsh:(bs + 1) * Dsh, 0] = (lnw_v if sle == 0 else lnw_i)[dsl]
            lw4[bs * Dsh:(bs + 1) * Dsh, 1] = (lnb_v if sle == 0 else lnb_i)[dsl]
        m["lnwb4"] = lw4
        o4 = np.zeros((128, C), NPBF16)
        for bs in range(2 * B):
            sle = bs % 2
            o4[bs * Dsh:(bs + 1) * Dsh] = (
                (opw_ms if sle == 0 else opw_pan)[:, dsl].T).astype(NPBF16)
        m["opw4"] = o4
        m["opw"] = np.ascontiguousarray(
            np.concatenate([opw_ms[:, dsl].T, opw_pan[:, dsl].T], 1)
        ).astype(NPBF16)
        maps.append(m)
    return maps


def mm(nc, out_ps, lhsT, rhs, start, stop, maxf=MMF):
    F = rhs.shape[-1]
    if F <= maxf:
        nc.tensor.matmul(out_ps, lhsT, rhs, start=start, stop=stop,
                         skip_group_check=True)
        return
    assert F % maxf == 0
    for i in range(F // maxf):
        nc.tensor.matmul(out_ps[:, i * maxf:(i + 1) * maxf], lhsT,
                         rhs[:, i * maxf:(i + 1) * maxf], start=start,
                         stop=stop, skip_group_check=True)


def rep_dram(tensor_ap, offset, row_stride, n_rows, n_rep, fstep, fcount):
    """DRAM src AP: dest p = rep*n_rows ... tiled replication
    (rep, row, f) -> [[0, n_rep], [row_stride, n_rows], [fstep, fcount]]."""
    return bass.AP(tensor=tensor_ap.tensor, offset=tensor_ap.offset + offset,
                   ap=[[0, n_rep], [row_stride, n_rows], [fstep, fcount]])


def rep_sbuf(ap2d, n_inner):
    """SBUF [P, F] -> dest [P*n_inner, F], dest p = i*n_inner+j reads src i."""
    pairs = [list(x) for x in ap2d.ap]
    assert len(pairs) == 2
    return bass.AP(tensor=ap2d.tensor, offset=ap2d.offset,
                   ap=[pairs[0], [0, n_inner], pairs[1]])


# ---------------------------------------------------------------------------
# Bass program
# ---------------------------------------------------------------------------

def build_nc(cfg: Cfg):
    B, C, H, W = cfg.B, cfg.C, cfg.H, cfg.W
    D, N, R, K = cfg.D, cfg.N, cfg.R, cfg.K
    Dsh, G, DG = cfg.Dsh, cfg.G, cfg.DG
    L, L2, Tc = cfg.L, cfg.L2, cfg.Tc
    XD = R + 2 * N                   # 40
    Hp, Wp = H + 2, W + 2
    NCH = L2 // Tc                   # 4 scan chunks
    TcL = 512                        # L-phase chunk
    nLch = L // TcL
    SC = 2 * B * L // 64
    groups = [list(range(cfg.n_cores))]
    half = 2 * B * L

    nc = bacc.Bacc("TRN2", target_bir_lowering=False, debug=False,
                   enable_asserts=False, num_devices=cfg.n_cores)

    t = {}

    def inp(name, shape, dt):
        t[name] = nc.dram_tensor(name, shape, dt, kind="ExternalInput").ap()

    inp("msf", [B, C, L], BF16)
    inp("panf", [B, C, L], BF16)
    inp("w_in_ms", [C, 2 * Dsh], BF16)
    inp("w_in_pan", [C, 2 * Dsh], BF16)
    inp("conv_d4", [128, 9 * 128], BF16)
    inp("conv_b4", [128, 1], F32)
    inp("xproj_T", [Dsh, K * XD], BF16)
    inp("dtw_pack", [K * R, K * Dsh], BF16)
    inp("dtb_pack", [K * Dsh, 1], F32)
    inp("A_col", [128, K * G], F32)
    inp("Dsum", [Dsh, 1], F32)
    inp("selg", [K * Dsh, G * 128], BF16)
    inp("red8", [128, G * Dsh], BF16)
    inp("selst", [128, 2 * 4 * B], BF16)
    inp("selbc", [2 * N, 2 * 128], BF16)
    inp("lnwb4", [128, 2], F32)
    inp("opw4", [128, C], BF16)
    inp("opw", [Dsh, 2 * C], BF16)

    out_ms = nc.dram_tensor("out_ms", [B, C, H, W], BF16,
                            kind="ExternalOutput").ap()
    out_pan = nc.dram_tensor("out_pan", [B, C, H, W], BF16,
                             kind="ExternalOutput").ap()

    shsp = "Shared" if cfg.n_cores > 4 else "Local"
    xdbl_part = nc.dram_tensor("xdbl_part", [B, K * XD, L2], BF16,
                               kind="Internal").ap()
    xdbl_full = nc.dram_tensor("xdbl_full", [B, K * XD, L2], BF16,
                               kind="Internal", addr_space=shsp).ap()
    brc_dram = nc.dram_tensor("brc_dram", [2 * N, B, L2], BF16,
                              kind="Internal").ap()
    stats_part = nc.dram_tensor("stats_part", [2, 2 * B, L], F32,
                                kind="Internal").ap()
    stats_full = nc.dram_tensor("stats_full", [2, 2 * B, L], F32,
                                kind="Internal", addr_space=shsp).ap()
    ab_dram = nc.dram_tensor("ab_dram", [2, 2 * B, L], BF16,
                             kind="Internal").ap()

    def sb(name, shape, dt):
        return nc.alloc_sbuf_tensor(name, shape, dt).ap()

    w_in_ms = sb("w_in_ms_s", [C, 2 * Dsh], BF16)
    w_in_pan = sb("w_in_pan_s", [C, 2 * Dsh], BF16)
    conv_d4 = sb("conv_d4_s", [128, 9, 128], BF16)
    conv_b4 = sb("conv_b4_s", [128, 1], F32)
    xproj_T = sb("xproj_T_s", [Dsh, K, XD], BF16)
    dtw_pack = sb("dtw_pack_s", [K * R, K * Dsh], BF16)
    dtb_pack = sb("dtb_pack_s", [K * Dsh, 1], F32)
    A_col = sb("A_col_s", [128, K * G], F32)
    Dsum_s = sb("Dsum_s", [Dsh, 1], F32)
    selg_s = sb("selg_s", [K * Dsh, G, 128], BF16)
    red8_s = sb("red8_s", [128, G, Dsh], BF16)
    selst = sb("selst_s", [128, 2, 4 * B], BF16)
    selbc = sb("selbc_s", [2 * N, 2, 128], BF16)
    lnwb4 = sb("lnwb4_s", [128, 2], F32)
    opw4 = sb("opw4_s", [128, C], BF16)
    opw_s = sb("opw_s", [Dsh, 2, C], BF16)

    zs4 = sb("zs4_s", [128, L], BF16)
    carry = sb("carry_s", [128, K * G], F32)
    ysv = sb("ysv_s", [Dsh, L2], BF16)
    y1v = sb("y1v_s", [Dsh, L2], BF16)

    def allreduce(in_ap, out_ap):
        if cfg.fake_cc:
            nc.sync.dma_start(out=out_ap, in_=in_ap)
        else:
            nc.gpsimd.collective_compute(
                "AllReduce", OP.add, replica_groups=groups,
                ins=[in_ap.opt()], outs=[out_ap.opt()])

    with tile.TileContext(nc) as tc:
        from contextlib import ExitStack
        # ---- weight loads ----
        for dst, srcw in [
            (w_in_ms, t["w_in_ms"]), (w_in_pan, t["w_in_pan"]),
            (conv_d4, t["conv_d4"].rearrange("p (t q) -> p t q", q=128)),
            (conv_b4, t["conv_b4"]),
            (xproj_T, t["xproj_T"].rearrange("p (k x) -> p k x", x=XD)),
            (dtw_pack, t["dtw_pack"]), (dtb_pack, t["dtb_pack"]),
            (A_col, t["A_col"]), (Dsum_s, t["Dsum"]),
            (selg_s, t["selg"].rearrange("p (g q) -> p g q", q=128)),
            (red8_s, t["red8"].rearrange("p (j q) -> p j q", q=Dsh)),
            (selst, t["selst"].rearrange("p (x m) -> p x m", m=4 * B)),
            (selbc, t["selbc"].rearrange("p (x q) -> p x q", q=128)),
            (lnwb4, t["lnwb4"]), (opw4, t["opw4"]),
            (opw_s, t["opw"].rearrange("p (s c) -> p s c", c=C)),
        ]:
            nc.sync.dma_start(out=dst, in_=srcw)

        ydp_ctx = ExitStack()
        ydp_pool = ydp_ctx.enter_context(tc.tile_pool(name="ydpp", bufs=1))
        ydp = ydp_pool.tile([128, L], BF16, tag="ydp")
        big_ctx = ExitStack()
        big = big_ctx.enter_context(tc.tile_pool(name="big", bufs=1))
        inter = big.tile([Dsh, B, L2], BF16, tag="inter")
        ndt = big.tile([K * Dsh, B, L2], BF16, tag="ndt")

        # ================= Phase F: in_proj + conv + silu =================
        if "f" in cfg.phases:
          with tc.tile_pool(name="f_ps", bufs=1, space="PSUM") as f_ps, \
               tc.tile_pool(name="f_cv", bufs=2, space="PSUM") as f_cv, \
               tc.tile_pool(name="x_ps", bufs=2, space="PSUM") as x_ps, \
               tc.tile_pool(name="x_st", bufs=3) as x_st, \
               tc.tile_pool(name="f_src", bufs=2) as f_src, \
               tc.tile_pool(name="f_mt", bufs=1) as f_mt, \
               tc.tile_pool(name="f_pad", bufs=1) as f_pad:
            xpad4 = f_pad.tile([128, Hp, Wp], BF16, tag="xpad4")
            nc.vector.memset(xpad4, 0.0)
            TOK = cfg.TOK
            mts = []
            for q in range(4):
                b, s = q // 2, q % 2
                srcT = t["msf"] if s == 0 else t["panf"]
                mt = f_mt.tile([C, L], BF16, tag=f"msrc{q}")
                nc.sync.dma_start(out=mt, in_=srcT[b])
                mts.append(mt)
            rpc = TOK // W
            for j in range(L // TOK):
                js = slice(j * TOK, (j + 1) * TOK)
                # quadrant pairs packed into [64,TOK] PSUM tiles (PE out
                # base must be 0/32/64): one silu / copy per pair
                for h in range(2):
                    ps_z = f_ps.tile([2 * Dsh, TOK], F32, tag=f"psz{h}")
                    ps_x = f_ps.tile([2 * Dsh, TOK], F32, tag=f"psx{h}")
                    for qq in range(2):
                        q = 2 * h + qq
                        w_in = w_in_ms if q % 2 == 0 else w_in_pan
                        qs = slice(qq * Dsh, (qq + 1) * Dsh)
                        mm(nc, ps_z[qs, :], w_in[:, 0:Dsh],
                           mts[q][:, js], start=True, stop=True)
                        mm(nc, ps_x[qs, :], w_in[:, Dsh:2 * Dsh],
                           mts[q][:, js], start=True, stop=True)
                    zh = slice(2 * h * Dsh, (2 * h + 2) * Dsh)
                    if h == 0:
                        nc.scalar.activation(out=zs4[zh, js], in_=ps_z,
                                             func=AF.Silu)
                    else:
                        zstg = f_src.tile([2 * Dsh, TOK], BF16, tag="zstg")
                        nc.scalar.activation(out=zstg, in_=ps_z,
                                             func=AF.Silu)
                        nc.vector.tensor_copy(out=zs4[zh, js], in_=zstg)
                    nc.vector.tensor_copy(
                        out=xpad4[zh, 1 + j * rpc:1 + (j + 1) * rpc,
                                  1:1 + W],
                        in_=ps_x.rearrange("p (r w) -> p r w", w=W))
            def emit_x(b_, jj):
                # x_dbl partial for (b_, token-chunk jj): 3 k-blocks
                for k_ in range(K):
                    st = x_st.tile([XD, Tc], BF16, tag="xst")
                    for sub in range(Tc // MMF):
                        ss = slice(sub * MMF, (sub + 1) * MMF)
                        ps = x_ps.tile([XD, MMF], F32, tag="xps")
                        nc.tensor.matmul(
                            ps, xproj_T[:, k_, :],
                            inter[:, b_, jj * Tc:(jj + 1) * Tc][:, ss],
                            start=True, stop=True, skip_group_check=True)
                        nc.scalar.copy(out=st[:, ss], in_=ps)
                    nc.sync.dma_start(
                        out=xdbl_part[b_, k_ * XD:(k_ + 1) * XD,
                                      jj * Tc:(jj + 1) * Tc], in_=st)

            # conv: all 4 quadrants at once, 9 taps, 8 row-chunks;
            # x_dbl chunks + the per-b AllReduce interleave as soon as
            # the needed inter columns exist
            CRW = 8
            for j in range(H // CRW):
                cps = f_cv.tile([128, CRW * W], F32, tag="cps")
                for tap in range(9):
                    ky, kx = tap // 3, tap % 3
                    rhs = xpad4[:, ky + j * CRW: ky + (j + 1) * CRW,
                                kx:kx + W]
                    nc.tensor.matmul(cps, conv_d4[:, tap, :], rhs,
                                     start=(tap == 0), stop=(tap == 8),
                                     skip_group_check=True)
                # silu(+bias) full width, then DVE strided copies to inter
                stg4 = f_src.tile([128, CRW * W], BF16, tag="stg4")
                if cfg.silu_lut:
                    nc.scalar.activation(out=stg4, in_=cps, func=AF.Silu,
                                         bias=conv_b4)
                else:
                    sgc = f_src.tile([128, CRW * W], BF16, tag="sgc")
                    nc.scalar.activation(out=sgc, in_=cps, func=AF.Sigmoid,
                                         bias=conv_b4)
                    nc.vector.scalar_tensor_tensor(
                        out=stg4, in0=cps, scalar=conv_b4, in1=sgc,
                        op0=OP.add, op1=OP.mult)
                for q in range(4):
                    b, s = q // 2, q % 2
                    base = inter[:, b, :]
                    ppair = list(base.ap[0])
                    dst = bass.AP(
                        tensor=base.tensor,
                        offset=base.offset + 2 * (j * CRW * W) + s,
                        ap=[ppair, [2, CRW * W]])
                    nc.vector.tensor_copy(
                        out=dst, in_=stg4[q * Dsh:(q + 1) * Dsh, :])
                if j % 2 == 1:
                    emit_x(0, j // 2)
                    if j == H // CRW - 1:
                        allreduce(xdbl_part[0], xdbl_full[0])
                    emit_x(1, j // 2)
                    if j == H // CRW - 1:
                        allreduce(xdbl_part[1], xdbl_full[1])

        # ================= Phase D: dt / dtx / reversals per b ===========
        if "d" in cfg.phases:
          with tc.tile_pool(name="d_ps", bufs=2, space="PSUM") as d_ps, \
               tc.tile_pool(name="d_xdt", bufs=1) as d_xdt, \
               tc.tile_pool(name="d_e", bufs=2) as d_e, \
               tc.tile_pool(name="d_bc", bufs=2) as d_bc:
            for b in range(B):
                xdt = d_xdt.tile([K * R, L2], BF16, tag="xdt")
                for k in range(K):
                    nc.sync.dma_start(
                        out=xdt[k * R:(k + 1) * R, :],
                        in_=xdbl_full[b, k * XD:k * XD + R, :])
                for jj in range(L2 // Tc):
                    ps = d_ps.tile([K * Dsh, Tc], F32, tag="dps")
                    mm(nc, ps, dtw_pack, xdt[:, jj * Tc:(jj + 1) * Tc],
                       start=True, stop=True)
                    e_t = d_e.tile([K * Dsh, Tc], F32, tag="et")
                    nc.scalar.activation(out=e_t, in_=ps, func=AF.Exp,
                                         bias=dtb_pack)
                    nc.scalar.activation(
                        out=ndt[:, b, jj * Tc:(jj + 1) * Tc], in_=e_t,
                        func=AF.Ln, bias=1.0)
                # reversed B/C rows for k=1 -> brc_dram (chunked)
                for jj in range(L2 // Tc):
                    bc1 = d_bc.tile([2 * N, Tc], BF16, tag="bc1")
                    nc.sync.dma_start(
                        out=bc1,
                        in_=xdbl_full[b, XD + R:XD + R + 2 * N,
                                      L2 - (jj + 1) * Tc:L2 - jj * Tc])
                    bcR = d_bc.tile([2 * N, Tc], BF16, tag="bcR")
                    nc.vector.tensor_copy(out=bcR, in_=bc1[:, ::-1])
                    nc.sync.dma_start(
                        out=brc_dram[:, b, jj * Tc:(jj + 1) * Tc], in_=bcR)

        # ================= Phase S: selective scan ========================
        if "s" in cfg.phases:
          with tc.tile_pool(name="s_y", bufs=1, space="PSUM") as s_y, \
               tc.tile_pool(name="s_dr", bufs=2, space="PSUM") as s_dr, \
               tc.tile_pool(name="s_bc", bufs=2, space="PSUM") as s_bc, \
               tc.tile_pool(name="s_a", bufs=2) as s_a, \
               tc.tile_pool(name="s_b", bufs=2) as s_b, \
               tc.tile_pool(name="s_h", bufs=2) as s_h, \
               tc.tile_pool(name="s_hc", bufs=2) as s_hc, \
               tc.tile_pool(name="s_rep", bufs=2) as s_rep, \
               tc.tile_pool(name="s_xr", bufs=4) as s_xr, \
               tc.tile_pool(name="s_bcin", bufs=2) as s_bcin, \
               tc.tile_pool(name="s_io", bufs=2) as s_io, \
               tc.tile_pool(name="s_ior", bufs=1) as s_ior:
            mulctr = [0]

            def big_mul(out, in0, in1):
                mulctr[0] += 1
                if cfg.pool_frac and mulctr[0] % cfg.pool_frac == 0:
                    nc.gpsimd.tensor_tensor(out=out, in0=in0, in1=in1,
                                            op=OP.mult)
                else:
                    nc.vector.tensor_tensor(out=out, in0=in0, in1=in1,
                                            op=OP.mult)

            seq = [(b, ch, k) for b in range(B) for ch in range(NCH)
                   for k in (0, 2, 1)]
            bc_tiles = {}

            def emit_bc(idx):
                # B/C rows: one linear [2N,Tc] load, then PE select-matmul
                # replication + Scalar evac; emitted one k-section early so
                # the DVE muls never wait on it
                if idx >= len(seq) or idx in bc_tiles:
                    return
                b_, ch_, k_ = seq[idx]
                cs_ = slice(ch_ * Tc, (ch_ + 1) * Tc)
                bct = s_bcin.tile([2 * N, Tc], BF16, tag="bct")
                if k_ == 1:
                    nc.gpsimd.dma_start(out=bct, in_=brc_dram[:, b_, cs_])
                else:
                    nc.gpsimd.dma_start(
                        out=bct, in_=xdbl_full[b_, k_ * XD + R:
                                               k_ * XD + R + 2 * N, cs_])
                B_rep = s_rep.tile([128, Tc], BF16, tag="brep")
                C_rep = s_rep.tile([128, Tc], BF16, tag="crep")
                for sub in range(Tc // MMF):
                    ss = slice(sub * MMF, (sub + 1) * MMF)
                    bps = s_bc.tile([128, MMF], F32, tag="bcps")
                    nc.tensor.matmul(
                        bps, selbc[:, 0, :], bct[:, ss],
                        start=True, stop=True, skip_group_check=True)
                    nc.scalar.copy(out=B_rep[:, ss], in_=bps)
                    cps2 = s_bc.tile([128, MMF], F32, tag="bcps")
                    nc.tensor.matmul(
                        cps2, selbc[:, 1, :], bct[:, ss],
                        start=True, stop=True, skip_group_check=True)
                    nc.scalar.copy(out=C_rep[:, ss], in_=cps2)
                bc_tiles[idx] = (B_rep, C_rep)

            emit_bc(0)
            idx = -1
            for b in range(B):
                for ch in range(NCH):
                    cs = slice(ch * Tc, (ch + 1) * Tc)
                    y_ps = s_y.tile([Dsh, Tc], F32, tag="yps")
                    for k in (0, 2, 1):
                        idx += 1
                        if k == 1:
                            # y02 complete: evac y0+y2+D*x to SBUF staging
                            nc.vector.scalar_tensor_tensor(
                                out=ysv[:, cs], in0=inter[:, b, cs],
                                scalar=Dsum_s[:, 0:1], in1=y_ps,
                                op0=OP.mult, op1=OP.add)
                            y_ps = s_y.tile([Dsh, Tc], F32, tag="yps")
                        blk = {0: 0, 2: 1, 1: 2}[k]
                        kb = 0 if k == 1 else blk * Dsh
                        if k == 1:
                            mcs = slice(L2 - (ch + 1) * Tc, L2 - ch * Tc)
                            src_dt = s_ior.tile([Dsh, Tc], BF16, tag="ndtRc")
                            nc.vector.tensor_copy(
                                out=src_dt,
                                in_=ndt[2 * Dsh:3 * Dsh, b, mcs][:, ::-1])
                            src_dtx = s_io.tile([Dsh, Tc], BF16, tag="dtxc")
                            nc.vector.tensor_tensor(
                                out=src_dtx, in0=src_dt,
                                in1=inter[:, b, mcs][:, ::-1], op=OP.mult)
                        else:
                            src_dt = ndt[blk * Dsh:(blk + 1) * Dsh, b, cs]
                            dt0 = src_dt
                            if blk != 0:
                                dt0 = s_ior.tile([Dsh, Tc], BF16, tag="dt0")
                                nc.vector.tensor_copy(out=dt0, in_=src_dt)
                            src_dtx = s_io.tile([Dsh, Tc], BF16, tag="dtxc")
                            nc.vector.tensor_tensor(
                                out=src_dtx, in0=dt0,
                                in1=inter[:, b, cs], op=OP.mult)
                        # issue all dtx replication DMAs up front so the
                        # g-loop never waits on them
                        dtxreps = []
                        for g in range(G):
                            dtxrep = s_xr.tile([128, Tc], BF16,
                                               tag="dtxrep")
                            nc.sync.dma_start(
                                out=dtxrep,
                                in_=rep_sbuf(src_dtx[g * DG:(g + 1) * DG],
                                             N))
                            dtxreps.append(dtxrep)
                        B_rep, C_rep = bc_tiles.pop(idx)
                        for g in range(G):
                            if g == 1:
                                emit_bc(idx + 1)
                            ci = blk * G + g
                            # a = exp(A * dt_rep); b = dtx_rep * B_rep
                            a_t = s_a.tile([128, Tc], BF16, tag="a")
                            b_t = s_b.tile([128, Tc], BF16, tag="b")
                            dtxrep = dtxreps[g]
                            if cfg.drps_big:
                                drps = s_dr.tile([128, Tc], F32, tag="drps")
                                mm(nc, drps, selg_s[kb:kb + Dsh, g, :],
                                   src_dt, start=True, stop=True)
                                nc.scalar.activation(
                                    out=a_t, in_=drps, func=AF.Exp,
                                    scale=A_col[:, ci:ci + 1])
                            else:
                                for sub in range(Tc // MMF):
                                    ss = slice(sub * MMF, (sub + 1) * MMF)
                                    drps = s_dr.tile([128, MMF], F32,
                                                     tag="drps")
                                    nc.tensor.matmul(
                                        drps, selg_s[kb:kb + Dsh, g, :],
                                        src_dt[:, ss],
                                        start=True, stop=True,
                                        skip_group_check=True)
                                    nc.scalar.activation(
                                        out=a_t[:, ss], in_=drps,
                                        func=AF.Exp,
                                        scale=A_col[:, ci:ci + 1])
                            big_mul(b_t, dtxrep, B_rep)
                            # scan
                            h_t = s_h.tile([128, Tc], BF16, tag="h")
                            init = 0.0 if ch == 0 else carry[:, ci:ci + 1]
                            nc.vector.tensor_tensor_scan(
                                h_t, a_t, b_t, init, OP.mult, OP.add)
                            if ch < NCH - 1:
                                if cfg.carry_eng == "pool":
                                    nc.gpsimd.tensor_copy(
                                        out=carry[:, ci:ci + 1],
                                        in_=h_t[:, Tc - 1:Tc])
                                else:
                                    nc.scalar.copy(
                                        out=carry[:, ci:ci + 1],
                                        in_=h_t[:, Tc - 1:Tc])
                            # hc = h * C_rep ; reduce over n into y_ps
                            hc = s_hc.tile([128, Tc], BF16, tag="hc")
                            big_mul(hc, h_t, C_rep)
                            mm(nc, y_ps, red8_s[:, g, :], hc,
                               start=(g == 0 and k in (0, 1)),
                               stop=(g == G - 1 and k in (2, 1)))
                    # y1 evac (base 0, no partition shift)
                    if cfg.y1_evac == "pool":
                        nc.gpsimd.tensor_copy(out=y1v[:, cs], in_=y_ps)
                    elif cfg.y1_evac == "act":
                        nc.scalar.copy(out=y1v[:, cs], in_=y_ps)
                    else:
                        nc.vector.tensor_copy(out=y1v[:, cs], in_=y_ps)
                # merge y02 with flipped y1 into ydp (inputs both base-0)
                for jj in range(NCH):
                    cs2 = slice(jj * Tc, (jj + 1) * Tc)
                    y1t = y1v[:, L2 - (jj + 1) * Tc:L2 - jj * Tc]
                    tok = slice(jj * Tc // 2, (jj + 1) * Tc // 2)
                    for par in range(2):
                        qs = (2 * b + par) * Dsh
                        nc.vector.tensor_tensor(
                            out=ydp[qs:qs + Dsh, tok],
                            in0=ysv[:, cs2][:, par::2],
                            in1=y1t[:, ::-1][:, par::2], op=OP.add)

        big_ctx.close()

        # ================= Phase L: LN + gate + out_proj ==================
        if "l" in cfg.phases:
          with tc.tile_pool(name="l_ps", bufs=2, space="PSUM") as l_ps, \
               tc.tile_pool(name="l_one", bufs=1) as l_one, \
               tc.tile_pool(name="l_sq", bufs=3) as l_sq, \
               tc.tile_pool(name="l_z", bufs=2 * 8) as l_z, \
               tc.tile_pool(name="l_stg", bufs=4) as l_stg:
            zwps, bzps = [], []
            for j in range(nLch):
                js = slice(j * TcL, (j + 1) * TcL)
                sqp = l_sq.tile([128, TcL], BF16, tag="sqp")
                nc.vector.tensor_tensor(out=sqp, in0=ydp[:, js],
                                        in1=ydp[:, js], op=OP.mult)
                sp = l_ps.tile([4 * B, TcL], F32, tag="sps")
                mm(nc, sp, selst[:, 0, :], ydp[:, js], start=True, stop=False)
                mm(nc, sp, selst[:, 1, :], sqp, start=False, stop=True)
                stg = l_stg.tile([4 * B, TcL], F32, tag="sstg2")
                nc.scalar.copy(out=stg, in_=sp)
                nc.sync.dma_start(
                    out=stats_part.rearrange("a x l -> (a x) l")[:, js],
                    in_=stg)
                # z-gate pre-products: independent of stats, fill DVE
                # while the stats AllReduce runs
                zwp = l_z.tile([128, TcL], BF16, tag="zwp")
                bzp = l_z.tile([128, TcL], BF16, tag="bzp")
                nc.vector.tensor_scalar_mul(zwp, zs4[:, js], lnwb4[:, 0:1])
                nc.vector.tensor_scalar_mul(bzp, zs4[:, js], lnwb4[:, 1:2])
                zwps.append(zwp)
                bzps.append(bzp)
            allreduce(stats_part, stats_full)
            s1f = l_one.tile([64, SC], F32, tag="s1f")
            s2f = l_one.tile([64, SC], F32, tag="s2f")
            flat = stats_full.rearrange("a x l -> (a x l)")
            nc.sync.dma_start(
                out=s1f, in_=flat[0:half].rearrange("(p c) -> p c", p=64))
            nc.sync.dma_start(
                out=s2f, in_=flat[half:2 * half].rearrange(
                    "(p c) -> p c", p=64))
            mu_t = l_one.tile([64, SC], F32, tag="mu_t")
            var_t = l_one.tile([64, SC], F32, tag="var_t")
            musq = l_one.tile([64, SC], F32, tag="musq")
            eps_t = l_one.tile([64, 1], F32, tag="eps_t")
            nc.vector.memset(eps_t, 1e-5)
            nc.vector.tensor_scalar_mul(mu_t, s1f, 1.0 / D)
            nc.vector.tensor_scalar_mul(var_t, s2f, 1.0 / D)
            nc.vector.tensor_mul(musq, mu_t, mu_t)
            nc.vector.tensor_sub(var_t, var_t, musq)
            nc.scalar.activation(out=var_t, in_=var_t, func=AF.Sqrt,
                                 bias=eps_t)
            nc.vector.reciprocal(out=s1f, in_=var_t)
            nc.vector.tensor_mul(s2f, mu_t, s1f)
            nc.vector.tensor_scalar_mul(s2f, s2f, -1.0)
            s1h = l_one.tile([64, SC], BF16, tag="s1h")
            s2h = l_one.tile([64, SC], BF16, tag="s2h")
            nc.vector.tensor_copy(out=s1h, in_=s1f)
            nc.vector.tensor_copy(out=s2h, in_=s2f)
            nc.sync.dma_start(
                out=ab_dram.rearrange("a x l -> (a x l)")[0:half].rearrange(
                    "(p c) -> p c", p=64), in_=s1h)
            nc.sync.dma_start(
                out=ab_dram.rearrange("a x l -> (a x l)")[
                    half:2 * half].rearrange("(p c) -> p c", p=64), in_=s2h)

            with tc.tile_pool(name="l_rep", bufs=3) as l_rep, \
                 tc.tile_pool(name="l_t", bufs=3) as l_t, \
                 tc.tile_pool(name="o_st", bufs=2) as o_st, \
                 tc.tile_pool(name="o_ps", bufs=2, space="PSUM") as o_ps:
                for j in range(nLch):
                    js = slice(j * TcL, (j + 1) * TcL)
                    zwp, bzp = zwps[j], bzps[j]
                    arp = l_rep.tile([128, TcL], BF16, tag="arp")
                    brp = l_rep.tile([128, TcL], BF16, tag="brp")
                    nc.sync.dma_start(
                        out=arp,
                        in_=bass.AP(tensor=ab_dram.tensor,
                                    offset=ab_dram.offset + j * TcL,
                                    ap=[[L, 2 * B], [0, Dsh], [1, TcL]]))
                    nc.scalar.dma_start(
                        out=brp,
                        in_=bass.AP(tensor=ab_dram.tensor,
                                    offset=ab_dram.offset + half + j * TcL,
                                    ap=[[L, 2 * B], [0, Dsh], [1, TcL]]))
                    t1 = l_t.tile([128, TcL], BF16, tag="t1")
                    nc.vector.tensor_mul(t1, ydp[:, js], arp)
                    t2 = l_t.tile([128, TcL], BF16, tag="t2")
                    nc.vector.tensor_add(t2, t1, brp)
                    t3 = l_t.tile([128, TcL], BF16, tag="t3")
                    nc.vector.tensor_mul(t3, t2, zwp)
                    fgp = l_t.tile([128, TcL], BF16, tag="fgp")
                    nc.vector.tensor_add(fgp, t3, bzp)
                    for bs in range(2 * B):
                        b, s = bs // 2, bs % 2
                        qs = slice(bs * Dsh, (bs + 1) * Dsh)
                        if bs < 3:
                            lhs = opw4[qs, :]
                            rhs = fgp[qs, :]
                        else:
                            stq = o_st.tile([Dsh, TcL], BF16, tag="stq")
                            nc.vector.tensor_copy(out=stq, in_=fgp[qs, :])
                            lhs = opw_s[:, s, :]
                            rhs = stq
                        ops = o_ps.tile([C, TcL], F32, tag="ops")
                        mm(nc, ops, lhs, rhs, start=True, stop=True)
                        ost = o_st.tile([C, TcL], BF16, tag="ost")
                        nc.scalar.copy(out=ost, in_=ops)
                        # bf16 partial output (summed in f32 on the host)
                        dst = out_ms if s == 0 else out_pan
                        nc.sync.dma_start(
                            out=dst[b].rearrange("c h w -> c (h w)")[:, js],
                            in_=ost)
        ydp_ctx.close()

    nc.compile()
    return nc


# ---------------------------------------------------------------------------

_CACHE = {}


def _get_nc(cfg: Cfg):
    if cfg not in _CACHE:
        _CACHE[cfg] = build_nc(cfg)
    return _CACHE[cfg]


def kernel(**inputs):
    cfg = CFG
    nc = _get_nc(cfg)
    in_maps = host_prep(cfg, inputs)
    res = bass_utils.run_bass_kernel_spmd(
        nc, in_maps, core_ids=list(range(cfg.n_cores)))
    return assemble_outputs(cfg, res.results)


def assemble_outputs(cfg, results):
    # each core emits a partial (its d_inner shard's contribution); sum.
    out_ms = np.zeros((cfg.B, cfg.C, cfg.H, cfg.W), np.float32)
    out_pan = np.zeros_like(out_ms)
    for r in range(cfg.n_cores):
        out_ms += np.asarray(results[r]["out_ms"], np.float32)
        out_pan += np.asarray(results[r]["out_pan"], np.float32)
    return (out_ms, out_pan)

